# revision 13
# baseline (speedup 1.0000x reference)
"""Trainium2 Bass kernel for nn_ApplyTimeChannel.

y[b,r,c,m] = sum_{a,l} h_time[b,r,c,0,a,m,l] * xp[b,0,a,g[m,l]]
with B=32, RX=1, RXA=16, TX=1, TXA=4, NT=2048, L=16, T=2063.

Strategy (data-parallel over batch, 4 batches per core, no collectives):
  host: gather xg = xp[..., g], pre-transpose h and xg so that SBUF
        partition p = (mh, a, l) with mh = which half of the padded
        2064-sample output-time axis, free dim mq (1032).
  dev:  h streams on the SWDGE queue (the ~310 GB/s/core DMA pool is
        shared across queues, so one fat stream; the first batch's
        first quad rides the scalar HWDGE ring which boots earlier).
        Per (b, block of c's): one fused DVE mul computes
        prod[p, (c, mq)] = h*xg in bf16 (v broadcast over the block);
        per c the PE contracts the 64-wide (a,l) axis per half with a
        [128, 32] ones stationary on PE column strip c%3
        (tile_position; quadrant 3 is unusable), writing psum rows
        32*(c%3) + 2*(c//3) + mh.  Strip classes stop at c=13/14/15,
        so evictions stagger and the tail stays short.  ACT evicts
        32-row strips -> bf16 SBUF; out DMAs ride the sync ring; the
        host un-permutes rows and casts to f32.
"""

import sys

if "/opt/trn_rl_repo" not in sys.path:
    sys.path.insert(0, "/opt/trn_rl_repo")

import numpy as np

B, C, A, NT, L, T = 32, 16, 4, 2048, 16, 2063
MH, MQ = 2, 1032  # padded T = 2064 = MH * MQ
P = 128  # partitions = MH * A * L
NCORES = 8
BS = B // NCORES  # batches per core
NQ = 4  # c-quads per batch (DMA + fused-mul granularity)
NBLK = ((0, 512), (512, 512), (1024, 8))  # mq -> psum bank blocks
GP_MULS = ()  # gpsimd tensor ops are 3.8x slower than DVE and stall the stream
SC_QUADS = ((1, 2), (2, 1))  # (b, q) quads DMA'd on the scalar HWDGE ring
HBUFS = 5
PBUFS = 4

TRACE = False
LAST = {}

_CACHE = {}

# last c of each strip class (c % 3): class 1 -> 13, class 2 -> 14, class 0 -> 15
_STRIP_LAST = {1: 13, 2: 14, 0: 15}


def _build_nc():
    import concourse.bacc as bacc
    import concourse.mybir as mybir
    import concourse.tile as tile

    f32 = mybir.dt.float32
    bf16 = mybir.dt.bfloat16

    nc = bacc.Bacc("TRN2", target_bir_lowering=False, debug=False)
    hh = nc.dram_tensor("hh", [BS, NQ, P, 4, MQ], bf16, kind="ExternalInput")
    vv = nc.dram_tensor("vv", [P, BS * MQ], bf16, kind="ExternalInput")
    ww = nc.dram_tensor("ww", [P, C * 32], bf16, kind="ExternalInput")
    out = nc.dram_tensor("out", [BS, 96, MQ], bf16, kind="ExternalOutput")

    from concourse.tile import add_dep_helper

    with tile.TileContext(nc) as tc:
        with (
            tc.tile_pool(name="wpool", bufs=1) as wpool,
            tc.tile_pool(name="vpool", bufs=1) as vpool,
            tc.tile_pool(name="hpool", bufs=HBUFS) as hpool,
            tc.tile_pool(name="ppool", bufs=PBUFS) as ppool,
            tc.tile_pool(name="cpool", bufs=4) as cpool,
            tc.tile_pool(name="ypool", bufs=6) as ypool,
            tc.tile_pool(name="pspool", bufs=6, space="PSUM") as pspool,
        ):
            # everything early rides the scalar HWDGE ring: it boots before
            # SWDGE and sustains 150+ GB/s (the sync ring only does ~25)
            vt = vpool.tile([P, BS, MQ], bf16, tag="v")
            nc.scalar.dma_start(out=vt[:], in_=vv[:])
            ht00 = hpool.tile([P, 4, MQ], bf16, tag="ht")
            nc.scalar.dma_start(out=ht00[:], in_=hh[0, 0])
            wb = wpool.tile([P, C * 32], bf16)
            nc.scalar.dma_start(out=wb[:], in_=ww[:])
            # dummy matmuls during the DMA-boot window keep the PE HAM
            # clock-gate open before the real matmuls arrive.
            wsc = wpool.tile([P, 32], bf16, tag="wsc")
            nc.vector.memset(wsc[:], 0)
            xsc = wpool.tile([P, 512], bf16, tag="xsc")
            nc.vector.memset(xsc[:], 0)
            pssc = pspool.tile([32, 512], f32, tag="pssc", bufs=1)
            warm_prev = None
            for i in range(14):
                wmm = nc.tensor.matmul(
                    out=pssc[:], lhsT=wsc[:], rhs=xsc[:], start=True, stop=True
                )
                if warm_prev is not None:
                    add_dep_helper(wmm.ins, warm_prev, sync=False,
                                   reason="warmup chain")
                warm_prev = wmm.ins

            def mms(pt_slice, c, psums):
                sc = c % 3
                for blk, (off, n) in enumerate(NBLK):
                    nc.tensor.matmul(
                        out=psums[blk][32 * sc : 32 * sc + 32, :],
                        lhsT=wb[:, c * 32 : (c + 1) * 32],
                        rhs=pt_slice[:, off : off + n],
                        start=(c < 3),
                        stop=(c >= 13),
                        skip_group_check=True,
                    )

            def evict_strip(b, s, psums, last=False):
                yt = ypool.tile([32, MQ], bf16, tag=f"y{s}")
                for blk, (off, n) in enumerate(NBLK):
                    nc.scalar.copy(
                        out=yt[:, off : off + n],
                        in_=psums[blk][32 * s : 32 * s + 32, :],
                    )
                nc.gpsimd.dma_start(out=out[b, 32 * s : 32 * s + 32, :], in_=yt[:])

            for b in range(BS):
                psums = [
                    pspool.tile([96, n], f32, tag="psum", name=f"ps{b}_{i}")
                    for i, (_, n) in enumerate(NBLK)
                ]

                fine_tail = b == BS - 1

                for q in range(NQ):
                    if fine_tail and q == NQ - 1:
                        break
                    if b == 0 and q == 0:
                        ht = ht00
                    else:
                        ht = hpool.tile([P, 4, MQ], bf16, tag="ht")
                        eng = nc.scalar if (b, q) in SC_QUADS else nc.gpsimd
                        eng.dma_start(out=ht[:], in_=hh[b, q])
                    pt = ppool.tile([P, 4, MQ], bf16)
                    eng = nc.gpsimd if (b, q) in GP_MULS else nc.vector
                    eng.tensor_mul(
                        out=pt[:],
                        in0=ht[:],
                        in1=vt[:, b : b + 1, :].broadcast_to([P, 4, MQ]),
                    )
                    for qc in range(4):
                        mms(pt[:, qc, :], 4 * q + qc, psums)

                if fine_tail:
                    # last quad: per-c DMAs + muls so the exposed tail after
                    # the final h bytes is one thin c-slice of work; strip
                    # evictions stagger at c=13/14/15.
                    for c in range(12, 16):
                        htc = cpool.tile([P, MQ], bf16, tag="htc")
                        nc.gpsimd.dma_start(
                            out=htc[:], in_=hh[b, 3, :, c - 12, :]
                        )
                        ptc = cpool.tile([P, MQ], bf16, tag="ptc")
                        nc.vector.tensor_mul(
                            out=ptc[:], in0=htc[:], in1=vt[:, b, :]
                        )
                        mms(ptc[:], c, psums)
                        if c >= 13:
                            evict_strip(b, c % 3, psums, last=True)
                else:
                    for s in (1, 2, 0):
                        evict_strip(b, s, psums)

    nc.compile()
    return nc


def _get_nc():
    if "nc" not in _CACHE:
        _CACHE["nc"] = _build_nc()
    return _CACHE["nc"]


def _make_ww():
    import ml_dtypes
    # c's stationary lives at columns [c*32, (c+1)*32); within it the ones
    # block for half mh sits at column 2*(c//3) + mh, so c's result lands
    # in psum rows 32*(c%3) + 2*(c//3) + mh of the strip-c%3 sub-array.
    ww = np.zeros((P, C * 32), np.float32)
    for c in range(C):
        for mh in range(MH):
            ww[mh * 64 : (mh + 1) * 64, c * 32 + 2 * (c // 3) + mh] = 1.0
    return ww.astype(ml_dtypes.bfloat16)


def _prep_inputs(x, h_time, g):
    import ml_dtypes

    x = np.asarray(x, dtype=np.float32)
    h = np.asarray(h_time, dtype=np.float32)
    g = np.asarray(g)

    # host gather: xg[b, a, m, l] = xp[b, a, g[m, l]]
    xsq = x.reshape(B, A, NT)
    xp = np.zeros((B, A, NT + 1), np.float32)
    xp[:, :, :NT] = xsq
    gi = np.clip(g.astype(np.int64), 0, NT)
    xg = xp[:, :, gi]  # [B, A, T, L]

    xgp = np.zeros((B, A, MH * MQ, L), np.float32)
    xgp[:, :, :T] = xg
    vvb = xgp.reshape(B, A, MH, MQ, L).transpose(0, 2, 1, 4, 3).reshape(B, P, MQ)
    vv = (
        vvb.reshape(NCORES, BS, P, MQ)
        .transpose(0, 2, 1, 3)
        .reshape(NCORES, P, BS * MQ)
    )
    vv = np.ascontiguousarray(vv).astype(ml_dtypes.bfloat16)

    hsq = h.reshape(B, C, A, T, L)
    hp = np.zeros((B, C, A, MH * MQ, L), np.float32)
    hp[:, :, :, :T] = hsq
    hh = (
        hp.reshape(B, C, A, MH, MQ, L)
        .transpose(0, 3, 2, 5, 1, 4)
        .reshape(B, P, C, MQ)
    )
    # [B, P, C, MQ] -> [B, NQ, P, 4, MQ]
    hh = hh.reshape(B, P, NQ, 4, MQ).transpose(0, 2, 1, 3, 4)
    hh = np.ascontiguousarray(hh).astype(ml_dtypes.bfloat16)
    return hh, vv, _make_ww()


_ROWS = None


def _row_map():
    global _ROWS
    if _ROWS is None:
        c = np.arange(C)
        _ROWS = (32 * (c % 3)[:, None] + 2 * (c // 3)[:, None]
                 + np.arange(MH)[None, :])  # [C, MH]
    return _ROWS


def _postprocess(res_list):
    # per-core out: [BS, 96, MQ] bf16; used rows per (c, mh) via _row_map
    y = np.concatenate(
        [np.asarray(r["out"]).astype(np.float32) for r in res_list], axis=0
    )  # [B, 96, MQ]
    y = y[:, _row_map(), :]  # [B, C, MH, MQ]
    y = y.reshape(B, C, MH * MQ)[:, :, :T]
    return np.ascontiguousarray(y.reshape(B, 1, C, T))


def kernel(x, h_time, g):
    from concourse.bass_utils import run_bass_kernel_spmd

    hh, vv, ww = _prep_inputs(x, h_time, g)
    in_maps = []
    for i in range(NCORES):
        sl = slice(i * BS, (i + 1) * BS)
        in_maps.append({"hh": hh[sl], "vv": vv[i], "ww": ww})

    nc = _get_nc()
    kw = {}
    if TRACE and LAST.get("trace_cores"):
        kw["trace_cores"] = LAST["trace_cores"]
    res = run_bass_kernel_spmd(
        nc, in_maps, core_ids=list(range(NCORES)), trace=TRACE, **kw
    )
    LAST["exec_time_ns"] = res.exec_time_ns
    LAST["result"] = res
    return _postprocess(res.results)


# revision 14
# speedup vs baseline: 1.2672x; 1.2672x over previous
"""v3 reference re-run: original baseline structure with bf16 h in DRAM."""

import sys

if "/opt/trn_rl_repo" not in sys.path:
    sys.path.insert(0, "/opt/trn_rl_repo")

import numpy as np

B, C, A, NT, L, T = 32, 16, 4, 2048, 16, 2063
MH, MQ = 2, 1032  # padded T = 2064 = MH * MQ
P = 128  # partitions = MH * A * L
NCORES = 8
BS = B // NCORES  # batches per core
NBLK = ((0, 512), (512, 512), (1024, 8))  # mq -> psum bank blocks
CBLK = 4  # c's per h DMA (2.1 MB transfers)
HBUFS = 5
PBUFS = 8

TRACE = False
LAST = {}

_CACHE = {}


def _build_nc():
    import concourse.bacc as bacc
    import concourse.mybir as mybir
    import concourse.tile as tile

    f32 = mybir.dt.float32
    bf16 = mybir.dt.bfloat16

    nc = bacc.Bacc("TRN2", target_bir_lowering=False, debug=False)
    hh = nc.dram_tensor("hh", [BS, P, C, MQ], bf16, kind="ExternalInput")
    vv = nc.dram_tensor("vv", [BS, P, MQ], bf16, kind="ExternalInput")
    ww = nc.dram_tensor("ww", [P, C * 32], bf16, kind="ExternalInput")
    out = nc.dram_tensor("out", [BS, 2 * C, MQ], f32, kind="ExternalOutput")

    from concourse.tile import add_dep_helper

    with tile.TileContext(nc) as tc:
        with (
            tc.tile_pool(name="wpool", bufs=1) as wpool,
            tc.tile_pool(name="vpool", bufs=BS) as vpool,
            tc.tile_pool(name="hpool", bufs=HBUFS) as hpool,
            tc.tile_pool(name="ppool", bufs=PBUFS) as ppool,
            tc.tile_pool(name="ypool", bufs=2) as ypool,
            tc.tile_pool(name="pspool", bufs=6, space="PSUM") as pspool,
        ):
            wb = wpool.tile([P, C * 32], bf16)
            nc.scalar.dma_start(out=wb[:], in_=ww[:])
            wsc = wpool.tile([P, 32], bf16, tag="wsc")
            nc.vector.memset(wsc[:], 0)
            xsc = wpool.tile([P, 512], bf16, tag="xsc")
            nc.vector.memset(xsc[:], 0)
            pssc = pspool.tile([32, 512], f32, tag="pssc", bufs=1)
            warm_prev = None
            for i in range(18):
                wmm = nc.tensor.matmul(
                    out=pssc[:], lhsT=wsc[:], rhs=xsc[:], start=True, stop=True
                )
                if warm_prev is not None:
                    add_dep_helper(wmm.ins, warm_prev, sync=False,
                                   reason="warmup chain")
                warm_prev = wmm.ins
            vts = []
            for b in range(BS):
                vt = vpool.tile([P, MQ], bf16, tag="v", name=f"v{b}")
                nc.sync.dma_start(out=vt[:], in_=vv[b])
                vts.append(vt)

            def cblocks(b):
                if b == BS - 1:
                    return [4, 4, 4, 2, 1, 1]
                return [CBLK] * (C // CBLK)

            for b in range(BS):
                psums = [
                    pspool.tile([2 * C, n], f32, tag="psum", name=f"ps{b}_{i}")
                    for i, (_, n) in enumerate(NBLK)
                ]

                def mms(pt, c, lo, hi):
                    for blk, (off, n) in enumerate(NBLK):
                        if off >= hi or off + n <= lo:
                            continue
                        nc.tensor.matmul(
                            out=psums[blk][:, :],
                            lhsT=wb[:, c * 32 : (c + 1) * 32],
                            rhs=pt[:, off : off + n],
                            start=(c == 0),
                            stop=(c == C - 1),
                        )

                c0 = 0
                for bi, nb in enumerate(cblocks(b)):
                    ht = hpool.tile([P, CBLK, MQ], bf16, tag="ht")
                    nc.gpsimd.dma_start(
                        out=ht[:, :nb, :], in_=hh[b, :, c0 : c0 + nb, :]
                    )
                    for cc in range(nb):
                        c = c0 + cc
                        pt = ppool.tile([P, MQ], bf16)
                        nc.vector.tensor_mul(out=pt[:], in0=ht[:, cc, :], in1=vts[b][:])
                        mms(pt, c, 0, MQ)
                    c0 += nb
                if b < BS - 1:
                    yt = ypool.tile([2 * C, MQ], f32)
                    for blk, (off, n) in enumerate(NBLK):
                        eng = nc.vector if blk == 1 else nc.scalar
                        if eng is nc.vector:
                            eng.tensor_copy(
                                out=yt[:, off : off + n], in_=psums[blk][:, :]
                            )
                        else:
                            eng.copy(out=yt[:, off : off + n], in_=psums[blk][:, :])
                    nc.scalar.dma_start(out=out[b], in_=yt[:])
                else:
                    y2 = ypool.tile([2 * C, 8], f32, tag="y2")
                    nc.scalar.copy(out=y2[:], in_=psums[2][:, :])
                    nc.sync.dma_start(out=out[b, :, 1024:MQ], in_=y2[:])
                    y0 = ypool.tile([2 * C, 512], f32, tag="y0")
                    nc.scalar.copy(out=y0[:], in_=psums[0][:, :])
                    nc.sync.dma_start(out=out[b, :, 0:512], in_=y0[:])
                    y1 = ypool.tile([2 * C, 512], f32, tag="y1")
                    nc.vector.tensor_copy(out=y1[:], in_=psums[1][:, :])
                    nc.scalar.dma_start(out=out[b, :, 512:1024], in_=y1[:])

    nc.compile()
    return nc


def _get_nc():
    if "nc" not in _CACHE:
        _CACHE["nc"] = _build_nc()
    return _CACHE["nc"]


def _make_ww():
    import ml_dtypes
    ww = np.zeros((P, C * 32), np.float32)
    for c in range(C):
        for mh in range(MH):
            ww[mh * 64 : (mh + 1) * 64, c * 32 + 2 * c + mh] = 1.0
    return ww.astype(ml_dtypes.bfloat16)


def _prep_inputs(x, h_time, g):
    x = np.asarray(x, dtype=np.float32)
    h = np.asarray(h_time, dtype=np.float32)
    g = np.asarray(g)

    xsq = x.reshape(B, A, NT)
    xp = np.zeros((B, A, NT + 1), np.float32)
    xp[:, :, :NT] = xsq
    gi = np.clip(g.astype(np.int64), 0, NT)
    xg = xp[:, :, gi]  # [B, A, T, L]

    xgp = np.zeros((B, A, MH * MQ, L), np.float32)
    xgp[:, :, :T] = xg
    import ml_dtypes
    vv = xgp.reshape(B, A, MH, MQ, L).transpose(0, 2, 1, 4, 3).reshape(B, P, MQ)
    vv = np.ascontiguousarray(vv).astype(ml_dtypes.bfloat16)

    hsq = h.reshape(B, C, A, T, L)
    hp = np.zeros((B, C, A, MH * MQ, L), np.float32)
    hp[:, :, :, :T] = hsq
    hh = (
        hp.reshape(B, C, A, MH, MQ, L)
        .transpose(0, 3, 2, 5, 1, 4)
        .reshape(B, P, C, MQ)
    )
    hh = np.ascontiguousarray(hh).astype(ml_dtypes.bfloat16)
    return hh, vv, _make_ww()


def _postprocess(res_list):
    y = np.concatenate([np.asarray(r["out"]) for r in res_list], axis=0)
    y = y.reshape(B, C, MH, MQ).reshape(B, C, MH * MQ)[:, :, :T]
    return np.ascontiguousarray(y.reshape(B, 1, C, T).astype(np.float32))


def kernel(x, h_time, g):
    from concourse.bass_utils import run_bass_kernel_spmd

    hh, vv, ww = _prep_inputs(x, h_time, g)
    in_maps = []
    for i in range(NCORES):
        sl = slice(i * BS, (i + 1) * BS)
        in_maps.append({"hh": hh[sl], "vv": vv[sl], "ww": ww})

    nc = _get_nc()
    kw = {}
    if TRACE and LAST.get("trace_cores"):
        kw["trace_cores"] = LAST["trace_cores"]
    res = run_bass_kernel_spmd(
        nc, in_maps, core_ids=list(range(NCORES)), trace=TRACE, **kw
    )
    LAST["exec_time_ns"] = res.exec_time_ns
    LAST["result"] = res
    return _postprocess(res.results)
